# revision 11
# baseline (speedup 1.0000x reference)
"""Trainium2 Bass kernel for nn_BinaryAttentionB (binary-quantised attention).

Contract: kernel(**inputs) takes the FULL unsharded inputs of
reference.setup_inputs() and returns the FULL output, computed on 8
NeuronCores.

Sharding: data-parallel over (batch, head-group): core = b*2 + g covers
batch b (of 4) and heads 3g..3g+2 (of 6).  The module's output
projection consumes o.reshape(B, S, 96) — a *pure row-major reshape* of
the per-batch [6, S, 16] attention output, so each core's 3 heads map to
exactly rows [g*1024, (g+1)*1024) of the final [2048, 384] output: cores
produce disjoint full output rows, no cross-core reduction.

The bernoulli draws of the reference's quantiser depend only on PRNG
keys and shapes (jax bernoulli = uniform(key, shape) < p), so the
uniform tensors are input-independent constants: generated on host CPU
once, shipped to the cores, compared against the on-device p.

Per-core device pipeline (all fp32):
  A) qkv projection (PE, K=384 accum over 3 chunks of 128) + tanh (ACT)
     + binary quantise (DVE) + PE-transpose of qs/ks into [64, S] layout.
  B) per head: scoresT tile [128 sk, 512 sq] = ksT^T.T @ qsT (one K=64
     matmul), attnT = exp(0.125*scores) (ACT, no max-subtraction needed:
     |scores| <= 8), then out2T [17, 512] += vaug^T.T @ attnT where vaug
     = [v | ones]: rows 0:16 are the unnormalised attention output
     (transposed), row 16 the softmax denominator.
  C) normalise, then the scrambled output projection: out rows [128,384]
     accumulate 6 matmuls with lhsT = strided (stride-6) slices of the
     [16, 6144] OT buffer — this implements the row-major reshape
     exactly, with Wd.T pre-arranged as [16, 6, 384] on host.
"""

import os
import sys

import numpy as np

for _p in ("/opt/trn_rl_repo", "/root/.axon_site/_ro/trn_rl_repo"):
    if os.path.isdir(_p) and _p not in sys.path:
        sys.path.append(_p)

B = 4
S = 2048
D = 384
H = 6
DH = 64
DV = 16
NHC = 3          # heads per core
NCORES = 8

TRACE = False    # test.py sets this for NTFF profiling
LAST_RESULTS = None

_CACHE = {}


# --------------------------------------------------------------------------
# Tile tail-drain workaround: this walrus build rejects CTRL instructions
# carrying >1 sem wait ("Too many sync wait commands").  Split the tail
# drain's waits across one Drain each.
# --------------------------------------------------------------------------
def _install_tile_patch():
    if _CACHE.get("tile_patched"):
        return
    import bass_rust
    import concourse.tile as tile
    from concourse.vector_clock import ScopedClock

    def _drain_and_barrier_split(self, tick_clock, wait_clock):
        nc = self.nc
        drain_inst = nc.sync.drain()
        wait_clock.add_sem_waits(
            drain_inst.ins, ScopedClock({None: tick_clock.global_clock})
        )
        si = drain_inst.ins.sync_info
        if si is not None and si.on_wait is not None and len(si.on_wait) > 1:
            waits = list(si.on_wait)
            drain_inst.ins.sync_info = bass_rust.SyncInfo(
                on_update=list(si.on_update or []), on_wait=[waits[0]]
            )
            for w in waits[1:]:
                d2 = nc.sync.drain()
                d2.ins.sync_info = bass_rust.SyncInfo(on_update=[], on_wait=[w])
        nc.all_engine_barrier()
        assert self.sems is not None
        popped = nc._tile_sem_poison_stack.pop()
        assert popped is self._sem_poison
        nc.clear_and_free_semaphores(list(self.sems.allocated().values()))
        nc.all_engine_barrier()

    tile.TileContext._drain_and_barrier = _drain_and_barrier_split
    _CACHE["tile_patched"] = True


def _split_multi_waits(nc):
    """Hoist all-but-one sem wait of any instruction onto standalone
    EventSemaphore instructions on the same engine, inserted just before
    it (same-stream waits execute in order, so semantics are identical)."""
    import bass_rust
    import concourse.mybir as mybir

    n = 0
    for fn in nc.m.functions:
        for blk in fn.blocks:
            insts = list(blk.instructions)
            out = []
            for inst in insts:
                si = getattr(inst, "sync_info", None)
                waits = list(si.on_wait) if si is not None and si.on_wait else []
                if len(waits) > 1:
                    for w in waits[:-1]:
                        n += 1
                        ev = mybir.InstEventSemaphore(
                            name=f"WSPLIT-{n}", ins=[], outs=[]
                        )
                        ev.engine = inst.engine
                        ev.sync_info = bass_rust.SyncInfo(
                            on_update=[], on_wait=[w]
                        )
                        out.append(ev)
                    inst.sync_info = bass_rust.SyncInfo(
                        on_update=list(si.on_update or []), on_wait=[waits[-1]]
                    )
                out.append(inst)
            if len(out) != len(insts):
                blk.instructions = out
    return n


def _install_ntff_hook():
    """Register the NTFF profiling hook (for TRACE mode) if absent."""
    if _CACHE.get("ntff_hooked"):
        return
    import types

    try:
        import antenv.axon_hooks  # noqa: F401
    except ImportError:
        mod = types.ModuleType("antenv.axon_hooks")
        _h = {}
        mod.set_axon_ntff_profile_hook = lambda h: _h.__setitem__("h", h)
        mod.get_axon_ntff_profile_hook = lambda: _h.get("h")
        sys.modules["antenv.axon_hooks"] = mod
        try:
            from trn_agent_boot.trn_boot import _ntff_profile_via_ctypes

            mod.set_axon_ntff_profile_hook(
                _ntff_profile_via_ctypes("/opt/axon/libaxon_pjrt.so")
            )
        except Exception:
            pass
    _CACHE["ntff_hooked"] = True


# --------------------------------------------------------------------------
# Device program
# --------------------------------------------------------------------------
def _build_nc(s_seq=S):
    import concourse.bass as bass
    import concourse.mybir as mybir
    import concourse.tile as tile
    from concourse.bass import ts
    from concourse.masks import make_identity

    _install_tile_patch()

    f32 = mybir.dt.float32
    add = mybir.AluOpType.add
    sub = mybir.AluOpType.subtract
    mult = mybir.AluOpType.mult
    is_lt = mybir.AluOpType.is_lt
    AF = mybir.ActivationFunctionType
    X = mybir.AxisListType.X

    n_t = s_seq // 128          # seq tiles of 128
    n_c4 = s_seq // 512         # seq chunks of 512
    n_st = s_seq * NHC // 6 // 128  # output row tiles
    oc = NHC * s_seq            # OT columns

    nc = bass.Bass()
    xT = nc.declare_dram_parameter("xT", [128, 3, s_seq], f32, isOutput=False)
    wAll = nc.declare_dram_parameter("wAll", [128, 3, 432], f32, isOutput=False)
    ball = nc.declare_dram_parameter("ball", [1, 432], f32, isOutput=False)
    wdT6 = nc.declare_dram_parameter("wdT6", [16, 6, 384], f32, isOutput=False)
    u_dram = {
        name: nc.declare_dram_parameter(name, [s_seq, 192], f32, isOutput=False)
        for name in ("u1q", "u2q", "u1k", "u2k")
    }
    out = nc.declare_dram_parameter(
        "out", [s_seq * NHC // 6, 384], f32, isOutput=True
    )

    with tile.TileContext(nc) as tc:
        with tc.tile_pool(name="singles", bufs=1) as singles:
            xT_sb = singles.tile([128, 3, s_seq], f32)
            nc.sync.dma_start(out=xT_sb, in_=xT[:])
            wAll_sb = singles.tile([128, 3, 432], f32)
            nc.sync.dma_start(out=wAll_sb, in_=wAll[:])
            ball_sb = singles.tile([128, 432], f32)
            nc.sync.dma_start(out=ball_sb, in_=ball[:].partition_broadcast(128))
            wdT6_sb = singles.tile([16, 6, 384], f32)
            nc.sync.dma_start(out=wdT6_sb, in_=wdT6[:])
            ident = singles.tile([128, 128], f32)
            make_identity(nc, ident)
            qsT_all = singles.tile([64, NHC, s_seq], f32)
            ksT_all = singles.tile([64, NHC, s_seq], f32)
            # cols 0:16 = v, cols 16:32 = 0, col 32 = ones (denominator row
            # must land on a 32-aligned PSUM partition for the DVE reads)
            vaug = [
                singles.tile([128, n_t, 33], f32, tag=f"vaug{h}", name=f"vaug{h}")
                for h in range(NHC)
            ]
            for h in range(NHC):
                nc.vector.memset(vaug[h][:, :, 16:32], 0.0)
                nc.vector.memset(vaug[h][:, :, 32:33], 1.0)
            ones16 = singles.tile([1, 16], f32)
            nc.vector.memset(ones16, 1.0)
            OT = singles.tile([16, oc], f32)

            # ---------------- Phase A: projections + quantise ----------------
            with (
                tc.tile_pool(name="pa", bufs=2) as pA,
                tc.tile_pool(name="pu", bufs=2) as pU,
                tc.tile_pool(name="psm", bufs=2) as pS,
                tc.tile_pool(name="ppA", bufs=2, space="PSUM") as ppA,
                tc.tile_pool(name="ppT", bufs=2, space="PSUM") as ppT,
            ):
                for t in range(n_t):
                    sl_t = ts(t, 128)
                    ps_qkv = ppA.tile([128, 432], f32)
                    for dk in range(3):
                        nc.tensor.matmul(
                            ps_qkv,
                            xT_sb[:, dk, sl_t],
                            wAll_sb[:, dk, :],
                            start=(dk == 0),
                            stop=(dk == 2),
                        )
                    qkv = pA.tile([128, 432], f32, tag="qkv")
                    nc.vector.tensor_tensor(
                        out=qkv, in0=ps_qkv, in1=ball_sb, op=add
                    )
                    th = pA.tile([128, 384], f32, tag="th")
                    nc.scalar.activation(out=th, in_=qkv[:, 0:384], func=AF.Tanh)
                    pp = pA.tile([128, 384], f32, tag="pp")
                    nc.vector.tensor_scalar(
                        out=pp, in0=th, scalar1=0.5, scalar2=0.5, op0=mult, op1=add
                    )
                    for pref, off in (("q", 0), ("k", 192)):
                        u1t = pU.tile([128, 192], f32, tag=f"u1{pref}")
                        nc.sync.dma_start(out=u1t, in_=u_dram[f"u1{pref}"][sl_t, :])
                        u2t = pU.tile([128, 192], f32, tag=f"u2{pref}")
                        nc.sync.dma_start(out=u2t, in_=u_dram[f"u2{pref}"][sl_t, :])
                        psl = pp[:, off : off + 192]
                        b1 = pA.tile([128, 192], f32, tag=f"b1{pref}")
                        nc.vector.tensor_tensor(out=b1, in0=u1t, in1=psl, op=is_lt)
                        b2 = pA.tile([128, 192], f32, tag=f"b2{pref}")
                        nc.vector.tensor_tensor(out=b2, in0=u2t, in1=psl, op=is_lt)
                        df1 = pA.tile([128, 192], f32, tag=f"df1{pref}")
                        nc.vector.tensor_tensor(out=df1, in0=psl, in1=b1, op=sub)
                        df2 = pA.tile([128, 192], f32, tag=f"df2{pref}")
                        nc.vector.tensor_tensor(out=df2, in0=psl, in1=b2, op=sub)
                        d1r = pS.tile([128, 3], f32, tag=f"d1r{pref}")
                        nc.vector.tensor_reduce(
                            out=d1r,
                            in_=df1.rearrange("p (h d) -> p h d", h=3),
                            axis=X,
                            op=add,
                            apply_absolute_value=True,
                        )
                        d2r = pS.tile([128, 3], f32, tag=f"d2r{pref}")
                        nc.vector.tensor_reduce(
                            out=d2r,
                            in_=df2.rearrange("p (h d) -> p h d", h=3),
                            axis=X,
                            op=add,
                            apply_absolute_value=True,
                        )
                        d1f = pS.tile([128, 3], f32, tag=f"d1f{pref}")
                        nc.vector.tensor_scalar(
                            out=d1f, in0=d1r, scalar1=1.0 / DH, scalar2=1e-12,
                            op0=mult, op1=add,
                        )
                        d2f = pS.tile([128, 3], f32, tag=f"d2f{pref}")
                        nc.vector.tensor_scalar(
                            out=d2f, in0=d2r, scalar1=1.0 / DH, scalar2=1e-12,
                            op0=mult, op1=add,
                        )
                        dsum = pS.tile([128, 3], f32, tag=f"ds{pref}")
                        nc.vector.tensor_tensor(out=dsum, in0=d1f, in1=d2f, op=add)
                        rec = pS.tile([128, 3], f32, tag=f"rc{pref}")
                        nc.vector.reciprocal(out=rec, in_=dsum)
                        w1 = pS.tile([128, 3], f32, tag=f"w1{pref}")
                        nc.vector.tensor_tensor(out=w1, in0=d2f, in1=rec, op=mult)
                        w2 = pS.tile([128, 3], f32, tag=f"w2{pref}")
                        nc.vector.tensor_tensor(out=w2, in0=d1f, in1=rec, op=mult)
                        a1 = pA.tile([128, 3, 64], f32, tag=f"a1{pref}")
                        nc.vector.tensor_tensor(
                            out=a1,
                            in0=b1.rearrange("p (h d) -> p h d", h=3),
                            in1=w1[:].broadcast_to([128, 3, 64]),
                            op=mult,
                        )
                        a2 = pA.tile([128, 3, 64], f32, tag=f"a2{pref}")
                        nc.vector.tensor_tensor(
                            out=a2,
                            in0=b2.rearrange("p (h d) -> p h d", h=3),
                            in1=w2[:].broadcast_to([128, 3, 64]),
                            op=mult,
                        )
                        qsp = pA.tile([128, 192], f32, tag=f"qsp{pref}")
                        nc.vector.tensor_tensor(
                            out=qsp.rearrange("p (h d) -> p h d", h=3),
                            in0=a1, in1=a2, op=add,
                        )
                        qs = pA.tile([128, 192], f32, tag=f"qs{pref}")
                        nc.vector.tensor_scalar(
                            out=qs, in0=qsp, scalar1=2.0, scalar2=-1.0,
                            op0=mult, op1=add,
                        )
                        dst_all = qsT_all if pref == "q" else ksT_all
                        for h in range(NHC):
                            psT = ppT.tile([64, 128], f32)
                            nc.tensor.transpose(psT, qs[:, ts(h, 64)], ident[:])
                            nc.scalar.copy(out=dst_all[:, h, sl_t], in_=psT)
                    for h in range(NHC):
                        nc.vector.tensor_copy(
                            out=vaug[h][:, t, 0:16],
                            in_=qkv[:, 384 + 16 * h : 384 + 16 * h + 16],
                        )

            # ---------------- Phase B: attention ----------------
            with (
                tc.tile_pool(name="pe", bufs=4) as pE,
                tc.tile_pool(name="pn", bufs=2) as pN,
                tc.tile_pool(name="ppS", bufs=3, space="PSUM") as ppS,
                tc.tile_pool(name="ppO", bufs=2, space="PSUM") as ppO,
                tc.tile_pool(name="ppB", bufs=2, space="PSUM") as ppB,
            ):
                for h in range(NHC):
                    qsT = qsT_all[:, h, :]
                    ksT = ksT_all[:, h, :]
                    for c4 in range(n_c4):
                        sl_q = ts(c4, 512)
                        psO = ppO.tile([33, 512], f32)
                        for tk in range(n_t):
                            psS = ppS.tile([128, 512], f32)
                            nc.tensor.matmul(
                                psS, ksT[:, ts(tk, 128)], qsT[:, sl_q],
                                start=True, stop=True,
                            )
                            eT = pE.tile([128, 512], f32, tag="eT")
                            nc.scalar.activation(
                                out=eT, in_=psS, func=AF.Exp, scale=0.125
                            )
                            nc.tensor.matmul(
                                psO, vaug[h][:, tk, :], eT,
                                start=(tk == 0), stop=(tk == n_t - 1),
                            )
                        den = pN.tile([1, 512], f32, tag="den")
                        nc.vector.tensor_copy(out=den, in_=psO[32:33, :])
                        psB = ppB.tile([16, 512], f32)
                        nc.tensor.matmul(psB, ones16[:], den[:], start=True, stop=True)
                        rbc = pN.tile([16, 512], f32, tag="rbc")
                        nc.vector.reciprocal(out=rbc, in_=psB)
                        nc.vector.tensor_tensor(
                            out=OT[:, h * s_seq + c4 * 512 : h * s_seq + c4 * 512 + 512],
                            in0=psO[0:16, :], in1=rbc, op=mult,
                        )

            # ---------------- Phase C: scrambled output projection ----------
            with (
                tc.tile_pool(name="pc", bufs=2) as pC,
                tc.tile_pool(name="ppC", bufs=2, space="PSUM") as ppC,
            ):
                OT_r = OT[:].rearrange("p (s six) -> p six s", six=6)
                for st in range(n_st):
                    psF = ppC.tile([128, 384], f32)
                    for t6 in range(6):
                        nc.tensor.matmul(
                            psF, OT_r[:, t6, ts(st, 128)], wdT6_sb[:, t6, :],
                            start=(t6 == 0), stop=(t6 == 5),
                        )
                    ob = pC.tile([128, 384], f32, tag="ob")
                    nc.vector.tensor_copy(out=ob, in_=psF)
                    nc.sync.dma_start(out=out[ts(st, 128), :], in_=ob)

    _split_multi_waits(nc)
    return nc


# --------------------------------------------------------------------------
# Host side
# --------------------------------------------------------------------------
def _uniforms(s_seq=S):
    key = ("uniforms", s_seq)
    if key in _CACHE:
        return _CACHE[key]
    import jax
    import jax.numpy as jnp

    cpu = jax.devices("cpu")[0]
    with jax.default_device(cpu):
        rkey = jax.random.key(42)
        kq, kk = jax.random.split(rkey)
        k1q, k2q = jax.random.split(kq)
        k1k, k2k = jax.random.split(kk)
        shp = (B * H, S, DH)
        us = {
            "u1q": np.asarray(jax.random.uniform(k1q, shp, jnp.float32)),
            "u2q": np.asarray(jax.random.uniform(k2q, shp, jnp.float32)),
            "u1k": np.asarray(jax.random.uniform(k1k, shp, jnp.float32)),
            "u2k": np.asarray(jax.random.uniform(k2k, shp, jnp.float32)),
        }
    if s_seq != S:
        us = {k: v[:, :s_seq, :] for k, v in us.items()}
    _CACHE[key] = us
    return us


def _prep_in_maps(inputs, s_seq=S):
    x = np.asarray(inputs["x"], np.float32)
    Wq = np.asarray(inputs["Wq"], np.float32)
    Wk = np.asarray(inputs["Wk"], np.float32)
    Wv = np.asarray(inputs["Wv"], np.float32)
    Wd = np.asarray(inputs["Wd"], np.float32)
    bq = np.asarray(inputs["bq"], np.float32)
    bk = np.asarray(inputs["bk"], np.float32)
    bv = np.asarray(inputs["bv"], np.float32)
    us = _uniforms(s_seq)

    wdT6 = np.ascontiguousarray(
        Wd.T.reshape(6, 16, 384).transpose(1, 0, 2)
    )  # [16, 6, 384]

    in_maps = []
    for core in range(NCORES):
        b, g = core // 2, core % 2
        h0 = NHC * g
        xb = x[b, :s_seq, :]  # [s, 384]
        xT3 = np.ascontiguousarray(
            xb.T.reshape(3, 128, s_seq).transpose(1, 0, 2)
        )  # [128, 3, s]
        wcat = np.concatenate(
            [
                Wq[h0 * DH : (h0 + NHC) * DH, :],
                Wk[h0 * DH : (h0 + NHC) * DH, :],
                Wv[h0 * DV : (h0 + NHC) * DV, :],
            ],
            axis=0,
        )  # [432, 384]
        wAll = np.ascontiguousarray(
            wcat.T.reshape(3, 128, 432).transpose(1, 0, 2)
        )  # [128, 3, 432]
        ball = np.concatenate(
            [
                bq[h0 * DH : (h0 + NHC) * DH],
                bk[h0 * DH : (h0 + NHC) * DH],
                bv[h0 * DV : (h0 + NHC) * DV],
            ]
        ).reshape(1, 432)
        m = {
            "xT": xT3,
            "wAll": wAll,
            "ball": np.ascontiguousarray(ball),
            "wdT6": wdT6,
        }
        for name in ("u1q", "u2q", "u1k", "u2k"):
            u = us[name]
            m[name] = np.ascontiguousarray(
                np.concatenate(
                    [u[b * H + h0 + h] for h in range(NHC)], axis=1
                )
            )  # [s, 192]
        in_maps.append(m)
    return in_maps


def kernel(**inputs):
    global LAST_RESULTS
    from concourse.bass_utils import run_bass_kernel_spmd

    if TRACE:
        _install_ntff_hook()

    if "nc" not in _CACHE:
        _CACHE["nc"] = _build_nc(S)
    nc = _CACHE["nc"]

    in_maps = _prep_in_maps(inputs, S)
    res = run_bass_kernel_spmd(
        nc, in_maps, core_ids=list(range(NCORES)), trace=TRACE
    )
    LAST_RESULTS = res

    bd = np.asarray(inputs["bd"], np.float32)
    out = np.empty((B, S, D), np.float32)
    rows = S // 2
    for core in range(NCORES):
        b, g = core // 2, core % 2
        out[b, g * rows : (g + 1) * rows, :] = res.results[core]["out"]
    out += bd
    return (out,)


# revision 17
# speedup vs baseline: 1.3250x; 1.3250x over previous
"""Trainium2 Bass kernel for nn_BinaryAttentionB (binary-quantised attention).

Contract: kernel(**inputs) takes the FULL unsharded inputs of
reference.setup_inputs() and returns the FULL output, computed on 8
NeuronCores.

Sharding: data-parallel over (batch, head-group): core = b*2 + g covers
batch b (of 4) and heads 3g..3g+2 (of 6).  The module's output
projection consumes o.reshape(B, S, 96) — a *pure row-major reshape* of
the per-batch [6, S, 16] attention output, so each core's 3 heads map to
exactly rows [g*1024, (g+1)*1024) of the final [2048, 384] output: cores
produce disjoint full output rows, no cross-core reduction.

The bernoulli draws of the reference's quantiser depend only on PRNG
keys and shapes (jax bernoulli = uniform(key, shape) < p), so the
uniform tensors are input-independent constants: generated on host CPU
once, shipped to the cores, compared against the on-device p.

Per-core device pipeline (all fp32):
  A) qkv projection (PE, K=384 accum over 3 chunks of 128) + tanh (ACT)
     + binary quantise (DVE) + PE-transpose of qs/ks into [64, S] layout.
  B) per head: scoresT tile [128 sk, 512 sq] = ksT^T.T @ qsT (one K=64
     matmul), attnT = exp(0.125*scores) (ACT, no max-subtraction needed:
     |scores| <= 8), then out2T [17, 512] += vaug^T.T @ attnT where vaug
     = [v | ones]: rows 0:16 are the unnormalised attention output
     (transposed), row 16 the softmax denominator.
  C) normalise, then the scrambled output projection: out rows [128,384]
     accumulate 6 matmuls with lhsT = strided (stride-6) slices of the
     [16, 6144] OT buffer — this implements the row-major reshape
     exactly, with Wd.T pre-arranged as [16, 6, 384] on host.
"""

import os
import sys

import numpy as np

for _p in ("/opt/trn_rl_repo", "/root/.axon_site/_ro/trn_rl_repo"):
    if os.path.isdir(_p) and _p not in sys.path:
        sys.path.append(_p)

B = 4
S = 2048
D = 384
H = 6
DH = 64
DV = 16
NHC = 3          # heads per core
NCORES = 8

TRACE = False    # test.py sets this for NTFF profiling
LAST_RESULTS = None

_CACHE = {}


# --------------------------------------------------------------------------
# Tile tail-drain workaround: this walrus build rejects CTRL instructions
# carrying >1 sem wait ("Too many sync wait commands").  Split the tail
# drain's waits across one Drain each.
# --------------------------------------------------------------------------
def _install_tile_patch():
    if _CACHE.get("tile_patched"):
        return
    import bass_rust
    import concourse.tile as tile
    from concourse.vector_clock import ScopedClock

    def _drain_and_barrier_split(self, tick_clock, wait_clock):
        nc = self.nc
        drain_inst = nc.sync.drain()
        wait_clock.add_sem_waits(
            drain_inst.ins, ScopedClock({None: tick_clock.global_clock})
        )
        si = drain_inst.ins.sync_info
        if si is not None and si.on_wait is not None and len(si.on_wait) > 1:
            waits = list(si.on_wait)
            drain_inst.ins.sync_info = bass_rust.SyncInfo(
                on_update=list(si.on_update or []), on_wait=[waits[0]]
            )
            for w in waits[1:]:
                d2 = nc.sync.drain()
                d2.ins.sync_info = bass_rust.SyncInfo(on_update=[], on_wait=[w])
        nc.all_engine_barrier()
        assert self.sems is not None
        popped = nc._tile_sem_poison_stack.pop()
        assert popped is self._sem_poison
        nc.clear_and_free_semaphores(list(self.sems.allocated().values()))
        nc.all_engine_barrier()

    tile.TileContext._drain_and_barrier = _drain_and_barrier_split
    _CACHE["tile_patched"] = True


def _split_multi_waits(nc):
    """Hoist all-but-one sem wait of any instruction onto standalone
    EventSemaphore instructions on the same engine, inserted just before
    it (same-stream waits execute in order, so semantics are identical)."""
    import bass_rust
    import concourse.mybir as mybir

    n = 0
    for fn in nc.m.functions:
        for blk in fn.blocks:
            insts = list(blk.instructions)
            out = []
            for inst in insts:
                si = getattr(inst, "sync_info", None)
                waits = list(si.on_wait) if si is not None and si.on_wait else []
                if len(waits) > 1:
                    for w in waits[:-1]:
                        n += 1
                        ev = mybir.InstEventSemaphore(
                            name=f"WSPLIT-{n}", ins=[], outs=[]
                        )
                        ev.engine = inst.engine
                        ev.sync_info = bass_rust.SyncInfo(
                            on_update=[], on_wait=[w]
                        )
                        out.append(ev)
                    inst.sync_info = bass_rust.SyncInfo(
                        on_update=list(si.on_update or []), on_wait=[waits[-1]]
                    )
                out.append(inst)
            if len(out) != len(insts):
                blk.instructions = out
    return n


def _install_ntff_hook():
    """Register the NTFF profiling hook (for TRACE mode) if absent."""
    if _CACHE.get("ntff_hooked"):
        return
    import types

    try:
        import antenv.axon_hooks  # noqa: F401
    except ImportError:
        mod = types.ModuleType("antenv.axon_hooks")
        _h = {}
        mod.set_axon_ntff_profile_hook = lambda h: _h.__setitem__("h", h)
        mod.get_axon_ntff_profile_hook = lambda: _h.get("h")
        sys.modules["antenv.axon_hooks"] = mod
        try:
            from trn_agent_boot.trn_boot import _ntff_profile_via_ctypes

            mod.set_axon_ntff_profile_hook(
                _ntff_profile_via_ctypes("/opt/axon/libaxon_pjrt.so")
            )
        except Exception:
            pass
    _CACHE["ntff_hooked"] = True


# --------------------------------------------------------------------------
# Device program
# --------------------------------------------------------------------------
def _build_nc(s_seq=S, split_waits=True):
    import concourse.bass as bass
    import concourse.mybir as mybir
    import concourse.tile as tile
    from concourse.bass import ts
    from concourse.masks import make_identity

    _install_tile_patch()

    f32 = mybir.dt.float32
    bf16 = mybir.dt.bfloat16
    if os.environ.get("KBF16", "1") != "1":
        bf16 = f32
    add = mybir.AluOpType.add
    sub = mybir.AluOpType.subtract
    mult = mybir.AluOpType.mult
    is_lt = mybir.AluOpType.is_lt
    AF = mybir.ActivationFunctionType
    X = mybir.AxisListType.X

    n_t = s_seq // 128          # seq tiles of 128
    n_c4 = s_seq // 512         # seq chunks of 512
    n_st = s_seq * NHC // 6 // 128  # output row tiles
    oc = NHC * s_seq            # OT columns

    nc = bass.Bass()
    xT = nc.declare_dram_parameter("xT", [128, 3, s_seq], f32, isOutput=False)
    wAll = nc.declare_dram_parameter("wAll", [128, 3, 432], f32, isOutput=False)
    ball = nc.declare_dram_parameter("ball", [1, 432], f32, isOutput=False)
    wdT6 = nc.declare_dram_parameter("wdT6", [16, 6, 384], f32, isOutput=False)
    u_dram = {
        name: nc.declare_dram_parameter(name, [s_seq, 192], f32, isOutput=False)
        for name in ("u1q", "u2q", "u1k", "u2k")
    }
    out = nc.declare_dram_parameter(
        "out", [s_seq * NHC // 6, 384], f32, isOutput=True
    )

    with tile.TileContext(nc) as tc:
        with tc.tile_pool(name="singles", bufs=1) as singles:
            xT_sb = singles.tile([128, 3, s_seq], f32)
            nc.sync.dma_start(out=xT_sb, in_=xT[:])
            wAll_sb = singles.tile([128, 3, 432], f32)
            nc.sync.dma_start(out=wAll_sb, in_=wAll[:])
            ball_sb = singles.tile([128, 432], f32)
            nc.sync.dma_start(out=ball_sb, in_=ball[:].partition_broadcast(128))
            wdT6_sb = singles.tile([16, 6, 384], f32)
            nc.sync.dma_start(out=wdT6_sb, in_=wdT6[:])
            ident = singles.tile([128, 128], f32)
            make_identity(nc, ident)
            qsT_all = singles.tile([64, NHC, s_seq], bf16)
            ksT_all = singles.tile([64, NHC, s_seq], bf16)
            # cols 0:16 = v, cols 16:32 = 0, col 32 = ones (denominator row
            # must land on a 32-aligned PSUM partition for the DVE reads)
            vaug = [
                singles.tile([128, n_t, 33], bf16, tag=f"vaug{h}", name=f"vaug{h}")
                for h in range(NHC)
            ]
            for h in range(NHC):
                nc.vector.memset(vaug[h][:, :, 16:32], 0.0)
                nc.vector.memset(vaug[h][:, :, 32:33], 1.0)
            ones16 = singles.tile([1, 16], f32)
            nc.vector.memset(ones16, 1.0)
            OT = singles.tile([16, oc], f32)

            # ---------------- Phase A: projections + quantise ----------------
            with (
                tc.tile_pool(name="pa", bufs=2) as pA,
                tc.tile_pool(name="pu", bufs=2) as pU,
                tc.tile_pool(name="psm", bufs=2) as pS,
                tc.tile_pool(name="ppA", bufs=2, space="PSUM") as ppA,
                tc.tile_pool(name="ppT", bufs=2, space="PSUM") as ppT,
            ):
                for t in range(n_t):
                    sl_t = ts(t, 128)
                    ps_qkv = ppA.tile([128, 432], f32)
                    for dk in range(3):
                        nc.tensor.matmul(
                            ps_qkv,
                            xT_sb[:, dk, sl_t],
                            wAll_sb[:, dk, :],
                            start=(dk == 0),
                            stop=(dk == 2),
                        )
                    qkv = pA.tile([128, 432], f32, tag="qkv")
                    nc.vector.tensor_tensor(
                        out=qkv, in0=ps_qkv, in1=ball_sb, op=add
                    )
                    th = pA.tile([128, 384], f32, tag="th")
                    nc.scalar.activation(out=th, in_=qkv[:, 0:384], func=AF.Tanh)
                    for pref, off in (("q", 0), ("k", 192)):
                        u1t = pU.tile([128, 192], f32, tag=f"u1{pref}")
                        nc.sync.dma_start(out=u1t, in_=u_dram[f"u1{pref}"][sl_t, :])
                        u2t = pU.tile([128, 192], f32, tag=f"u2{pref}")
                        nc.sync.dma_start(out=u2t, in_=u_dram[f"u2{pref}"][sl_t, :])
                        psl = th[:, off : off + 192]
                        b1 = pA.tile([128, 192], f32, tag=f"b1{pref}")
                        nc.vector.tensor_tensor(out=b1, in0=u1t, in1=psl, op=is_lt)
                        b2 = pA.tile([128, 192], f32, tag=f"b2{pref}")
                        nc.vector.tensor_tensor(out=b2, in0=u2t, in1=psl, op=is_lt)
                        qb1 = pA.tile([128, 192], f32, tag=f"qb1{pref}")
                        nc.vector.tensor_scalar(
                            out=qb1, in0=b1, scalar1=2.0, scalar2=-1.0,
                            op0=mult, op1=add,
                        )
                        qb2 = pA.tile([128, 192], f32, tag=f"qb2{pref}")
                        nc.vector.tensor_scalar(
                            out=qb2, in0=b2, scalar1=2.0, scalar2=-1.0,
                            op0=mult, op1=add,
                        )
                        df1 = pA.tile([128, 192], f32, tag=f"df1{pref}")
                        nc.vector.tensor_tensor(out=df1, in0=psl, in1=qb1, op=sub)
                        df2 = pA.tile([128, 192], f32, tag=f"df2{pref}")
                        nc.vector.tensor_tensor(out=df2, in0=psl, in1=qb2, op=sub)
                        d1r = pS.tile([128, 3], f32, tag=f"d1r{pref}")
                        nc.vector.tensor_reduce(
                            out=d1r,
                            in_=df1.rearrange("p (h d) -> p h d", h=3),
                            axis=X,
                            op=add,
                            apply_absolute_value=True,
                        )
                        d2r = pS.tile([128, 3], f32, tag=f"d2r{pref}")
                        nc.vector.tensor_reduce(
                            out=d2r,
                            in_=df2.rearrange("p (h d) -> p h d", h=3),
                            axis=X,
                            op=add,
                            apply_absolute_value=True,
                        )
                        d1f = pS.tile([128, 3], f32, tag=f"d1f{pref}")
                        nc.vector.tensor_scalar(
                            out=d1f, in0=d1r, scalar1=0.5 / DH, scalar2=1e-12,
                            op0=mult, op1=add,
                        )
                        d2f = pS.tile([128, 3], f32, tag=f"d2f{pref}")
                        nc.vector.tensor_scalar(
                            out=d2f, in0=d2r, scalar1=0.5 / DH, scalar2=1e-12,
                            op0=mult, op1=add,
                        )
                        dsum = pS.tile([128, 3], f32, tag=f"ds{pref}")
                        nc.vector.tensor_tensor(out=dsum, in0=d1f, in1=d2f, op=add)
                        rec = pS.tile([128, 3], f32, tag=f"rc{pref}")
                        nc.vector.reciprocal(out=rec, in_=dsum)
                        w1 = pS.tile([128, 3], f32, tag=f"w1{pref}")
                        nc.vector.tensor_tensor(out=w1, in0=d2f, in1=rec, op=mult)
                        w2 = pS.tile([128, 3], f32, tag=f"w2{pref}")
                        nc.vector.tensor_tensor(out=w2, in0=d1f, in1=rec, op=mult)
                        a1 = pA.tile([128, 3, 64], f32, tag=f"a1{pref}")
                        nc.vector.tensor_tensor(
                            out=a1,
                            in0=qb1.rearrange("p (h d) -> p h d", h=3),
                            in1=w1[:].broadcast_to([128, 3, 64]),
                            op=mult,
                        )
                        a2 = pA.tile([128, 3, 64], f32, tag=f"a2{pref}")
                        nc.vector.tensor_tensor(
                            out=a2,
                            in0=qb2.rearrange("p (h d) -> p h d", h=3),
                            in1=w2[:].broadcast_to([128, 3, 64]),
                            op=mult,
                        )
                        qs = pA.tile([128, 192], f32, tag=f"qs{pref}")
                        nc.vector.tensor_tensor(
                            out=qs.rearrange("p (h d) -> p h d", h=3),
                            in0=a1, in1=a2, op=add,
                        )
                        dst_all = qsT_all if pref == "q" else ksT_all
                        for h in range(NHC):
                            psT = ppT.tile([64, 128], f32)
                            nc.tensor.transpose(psT, qs[:, ts(h, 64)], ident[:])
                            nc.scalar.copy(out=dst_all[:, h, sl_t], in_=psT)
                    for h in range(NHC):
                        nc.vector.tensor_copy(
                            out=vaug[h][:, t, 0:16],
                            in_=qkv[:, 384 + 16 * h : 384 + 16 * h + 16],
                        )

            # ---------------- Phase B: attention ----------------
            with (
                tc.tile_pool(name="pe", bufs=4) as pE,
                tc.tile_pool(name="pn", bufs=2) as pN,
                tc.tile_pool(name="ppS", bufs=3, space="PSUM") as ppS,
                tc.tile_pool(name="ppO", bufs=2, space="PSUM") as ppO,
                tc.tile_pool(name="ppB", bufs=2, space="PSUM") as ppB,
            ):
                for h in range(NHC):
                    qsT = qsT_all[:, h, :]
                    ksT = ksT_all[:, h, :]
                    for c4 in range(n_c4):
                        sl_q = ts(c4, 512)
                        psO = ppO.tile([33, 512], f32)
                        for tk in range(n_t):
                            psS = ppS.tile([128, 512], f32)
                            nc.tensor.matmul(
                                psS, ksT[:, ts(tk, 128)], qsT[:, sl_q],
                                start=True, stop=True,
                            )
                            eT = pE.tile([128, 512], bf16, tag="eT")
                            nc.scalar.activation(
                                out=eT, in_=psS, func=AF.Exp, scale=0.125
                            )
                            nc.tensor.matmul(
                                psO, vaug[h][:, tk, :], eT,
                                start=(tk == 0), stop=(tk == n_t - 1),
                            )
                        den = pN.tile([1, 512], f32, tag="den")
                        nc.vector.tensor_copy(out=den, in_=psO[32:33, :])
                        psB = ppB.tile([16, 512], f32)
                        nc.tensor.matmul(psB, ones16[:], den[:], start=True, stop=True)
                        rbc = pN.tile([16, 512], f32, tag="rbc")
                        nc.vector.reciprocal(out=rbc, in_=psB)
                        nc.vector.tensor_tensor(
                            out=OT[:, h * s_seq + c4 * 512 : h * s_seq + c4 * 512 + 512],
                            in0=psO[0:16, :], in1=rbc, op=mult,
                        )

            # ---------------- Phase C: scrambled output projection ----------
            with (
                tc.tile_pool(name="pc", bufs=2) as pC,
                tc.tile_pool(name="ppC", bufs=2, space="PSUM") as ppC,
            ):
                OT_r = OT[:].rearrange("p (s six) -> p six s", six=6)
                for st in range(n_st):
                    psF = ppC.tile([128, 384], f32)
                    for t6 in range(6):
                        nc.tensor.matmul(
                            psF, OT_r[:, t6, ts(st, 128)], wdT6_sb[:, t6, :],
                            start=(t6 == 0), stop=(t6 == 5),
                        )
                    ob = pC.tile([128, 384], f32, tag="ob")
                    nc.vector.tensor_copy(out=ob, in_=psF)
                    nc.sync.dma_start(out=out[ts(st, 128), :], in_=ob)

    if split_waits:
        _split_multi_waits(nc)
    return nc


# --------------------------------------------------------------------------
# Host side
# --------------------------------------------------------------------------
def _uniforms(s_seq=S):
    key = ("uniforms", s_seq)
    if key in _CACHE:
        return _CACHE[key]
    import jax
    import jax.numpy as jnp

    cpu = jax.devices("cpu")[0]
    with jax.default_device(cpu):
        rkey = jax.random.key(42)
        kq, kk = jax.random.split(rkey)
        k1q, k2q = jax.random.split(kq)
        k1k, k2k = jax.random.split(kk)
        shp = (B * H, S, DH)
        us = {
            "u1q": np.asarray(jax.random.uniform(k1q, shp, jnp.float32)),
            "u2q": np.asarray(jax.random.uniform(k2q, shp, jnp.float32)),
            "u1k": np.asarray(jax.random.uniform(k1k, shp, jnp.float32)),
            "u2k": np.asarray(jax.random.uniform(k2k, shp, jnp.float32)),
        }
    if s_seq != S:
        us = {k: v[:, :s_seq, :] for k, v in us.items()}
    _CACHE[key] = us
    return us


def _prep_in_maps(inputs, s_seq=S):
    x = np.asarray(inputs["x"], np.float32)
    Wq = np.asarray(inputs["Wq"], np.float32)
    Wk = np.asarray(inputs["Wk"], np.float32)
    Wv = np.asarray(inputs["Wv"], np.float32)
    Wd = np.asarray(inputs["Wd"], np.float32)
    bq = np.asarray(inputs["bq"], np.float32)
    bk = np.asarray(inputs["bk"], np.float32)
    bv = np.asarray(inputs["bv"], np.float32)
    us = _uniforms(s_seq)

    wdT6 = np.ascontiguousarray(
        Wd.T.reshape(6, 16, 384).transpose(1, 0, 2)
    )  # [16, 6, 384]

    in_maps = []
    for core in range(NCORES):
        b, g = core // 2, core % 2
        h0 = NHC * g
        xb = x[b, :s_seq, :]  # [s, 384]
        xT3 = np.ascontiguousarray(
            xb.T.reshape(3, 128, s_seq).transpose(1, 0, 2)
        )  # [128, 3, s]
        wcat = np.concatenate(
            [
                Wq[h0 * DH : (h0 + NHC) * DH, :],
                Wk[h0 * DH : (h0 + NHC) * DH, :],
                Wv[h0 * DV : (h0 + NHC) * DV, :],
            ],
            axis=0,
        )  # [432, 384]
        wAll = np.ascontiguousarray(
            wcat.T.reshape(3, 128, 432).transpose(1, 0, 2)
        )  # [128, 3, 432]
        ball = np.concatenate(
            [
                bq[h0 * DH : (h0 + NHC) * DH],
                bk[h0 * DH : (h0 + NHC) * DH],
                bv[h0 * DV : (h0 + NHC) * DV],
            ]
        ).reshape(1, 432)
        m = {
            "xT": xT3,
            "wAll": wAll,
            "ball": np.ascontiguousarray(ball),
            "wdT6": wdT6,
        }
        for name in ("u1q", "u2q", "u1k", "u2k"):
            u = us[name]
            m[name] = np.ascontiguousarray(
                np.concatenate(
                    [u[b * H + h0 + h] for h in range(NHC)], axis=1
                ) * 2.0 - 1.0
            )  # [s, 192], pre-mapped to tanh domain (b = 2u-1 < tanh)
        in_maps.append(m)
    return in_maps


def kernel(**inputs):
    global LAST_RESULTS
    from concourse.bass_utils import run_bass_kernel_spmd

    if TRACE:
        _install_ntff_hook()

    if "nc" not in _CACHE:
        _CACHE["nc"] = _build_nc(S)
    nc = _CACHE["nc"]

    in_maps = _prep_in_maps(inputs, S)
    res = run_bass_kernel_spmd(
        nc, in_maps, core_ids=list(range(NCORES)), trace=TRACE
    )
    LAST_RESULTS = res

    bd = np.asarray(inputs["bd"], np.float32)
    out = np.empty((B, S, D), np.float32)
    rows = S // 2
    for core in range(NCORES):
        b, g = core // 2, core % 2
        out[b, g * rows : (g + 1) * rows, :] = res.results[core]["out"]
    out += bd
    return (out,)
